# revision 13
# baseline (speedup 1.0000x reference)
"""MoD (mixture-of-depths) routing layer on 8 Trainium2 NeuronCores.

Reference computation (per token t of x[B=4, S=4096, D=1024]):
    logit[t] = x[t] @ W_r + b_r
    mask[t]  = sigmoid(logit[t]) > 0.5      (== logit[t] > 0)
    h[t]     = relu(x[t] @ W1 + b1)
    proc[t]  = h[t] @ W2 + b2
    out[t]   = mask[t] ? proc[t] : x[t]
    frac     = mean(mask)

Sharding: tokens (B*S = 16384) are split contiguously across the 8 cores
(2048 tokens each); router + MLP weights are replicated.  No cross-core
communication is needed; frac is assembled on host from per-core counts.

Device kernel (per core, T = 2048 tokens):
  * router on DVE in fp32 (exact products; min |logit| over the dataset is
    ~1.6e-4 so fp16 routing would flip tokens, fp32 will not)
  * MLP matmuls on the PE array in fp16 (fp32 PSUM accumulation)
  * mm1 computes h^T tiles (stationary W1), mm2 consumes h^T as the
    stationary side so the output lands token-major; relu+b1 fused into the
    PSUM eviction, b2 added via a broadcast tile
  * final select is a copy of x overwritten by proc where mask!=0
"""

import sys
import numpy as np

sys.path.insert(0, "/opt/trn_rl_repo")

from contextlib import ExitStack  # noqa: E402

import concourse.bass as bass  # noqa: E402
import concourse.bacc as bacc  # noqa: E402
import concourse.mybir as mybir  # noqa: E402
import concourse.tile as tile  # noqa: E402

FP32 = mybir.dt.float32
FP16 = mybir.dt.float16

N_CORES = 8
T = 2048          # tokens per core
D = 1024          # model dim
H = 4096          # hidden dim
NTT = T // 128    # token tiles per core (16)
NCH = 256         # tokens per mm chunk
NCHT = NCH // 128  # token tiles per chunk (2)
NCHUNK = T // NCH  # chunks per core (8)
KD = D // 128     # k-tiles over model dim (8)
KH = H // 128     # k-tiles over hidden dim (32)
MH = H // 128     # m-tiles over hidden dim (32)


def build_dense():
    nc = bacc.Bacc()

    x_in = nc.declare_dram_parameter("x", [T, D], FP32, isOutput=False)
    xt16_in = nc.declare_dram_parameter("xt16", [D, T], FP16, isOutput=False)
    w1_in = nc.declare_dram_parameter("w1", [D, H], FP16, isOutput=False)
    w2_in = nc.declare_dram_parameter("w2", [H, D], FP16, isOutput=False)
    b1_in = nc.declare_dram_parameter("b1", [H], FP32, isOutput=False)
    b2_in = nc.declare_dram_parameter("b2", [D], FP32, isOutput=False)
    wr_in = nc.declare_dram_parameter("wr", [D], FP32, isOutput=False)
    br_in = nc.declare_dram_parameter("br", [1], FP32, isOutput=False)
    out_ext = nc.declare_dram_parameter("out", [T, D], FP32, isOutput=True)
    cnt_ext = nc.declare_dram_parameter("cnt", [1, 1], FP32, isOutput=True)

    with tile.TileContext(nc) as tc, ExitStack() as ctx:
        const = ctx.enter_context(tc.tile_pool(name="const", bufs=1))
        wts = ctx.enter_context(tc.tile_pool(name="wts", bufs=1))
        xt_pool = ctx.enter_context(tc.tile_pool(name="xt", bufs=2))
        h_pool = ctx.enter_context(tc.tile_pool(name="h", bufs=1))
        xrt_pool = ctx.enter_context(tc.tile_pool(name="xrt", bufs=2))
        vec_pool = ctx.enter_context(tc.tile_pool(name="vec", bufs=2))
        out_pool = ctx.enter_context(tc.tile_pool(name="outp", bufs=2))
        p1 = ctx.enter_context(tc.tile_pool(name="p1", bufs=3, space="PSUM"))
        p2 = ctx.enter_context(tc.tile_pool(name="p2", bufs=4, space="PSUM"))
        pc = ctx.enter_context(tc.tile_pool(name="pc", bufs=1, space="PSUM"))

        # --- constants / broadcasts ---
        wr_bc = const.tile([128, D], FP32, tag="wr_bc")
        nc.sync.dma_start(out=wr_bc[:], in_=wr_in[None, :].partition_broadcast(128))
        b2_bc = const.tile([128, D], FP32, tag="b2_bc")
        nc.sync.dma_start(out=b2_bc[:], in_=b2_in[None, :].partition_broadcast(128))
        nbr_bc = const.tile([128, 1], FP32, tag="nbr_bc")
        nc.sync.dma_start(out=nbr_bc[:], in_=br_in[None, :].partition_broadcast(128))
        nc.vector.tensor_scalar_mul(nbr_bc[:], nbr_bc[:], -1.0)
        b1_sb = const.tile([128, MH], FP32, tag="b1_sb")
        nc.sync.dma_start(out=b1_sb[:], in_=b1_in.rearrange("(m p) -> p m", p=128))
        ones_col = const.tile([128, 1], FP32, tag="ones")
        nc.any.memset(ones_col[:], 1.0)

        # --- first chunk's activations, then W1, then W2 (W2 on the ACT
        # HWDGE queue so it doesn't delay the PE's first matmuls) ---
        xt_ch0 = []
        for k in range(KD):
            t_ = xt_pool.tile([128, NCH], FP16, tag=f"xt_{k}")
            nc.sync.dma_start(out=t_[:], in_=xt16_in[k * 128:(k + 1) * 128, 0:NCH])
            xt_ch0.append(t_)
        w1_sb = []
        for k in range(KD):
            t_ = wts.tile([128, H], FP16, tag=f"w1_{k}")
            nc.sync.dma_start(out=t_[:], in_=w1_in[k * 128:(k + 1) * 128, :])
            w1_sb.append(t_)
        w2_sb = []
        for k in range(KH):
            t_ = wts.tile([128, D], FP16, tag=f"w2_{k}")
            nc.scalar.dma_start(out=t_[:], in_=w2_in[k * 128:(k + 1) * 128, :])
            w2_sb.append(t_)

        # --- router: fp32 on DVE, token-major ---
        mask_nm = const.tile([128, NTT], FP32, tag="mask")
        mask_u = const.tile([128, NTT], mybir.dt.uint32, tag="mask_u")
        for n in range(NTT):
            x_t = xrt_pool.tile([128, D], FP32, tag="x_rt")
            nc.sync.dma_start(out=x_t[:], in_=x_in[n * 128:(n + 1) * 128, :])
            prod = vec_pool.tile([128, D], FP32, tag="prod")
            nc.vector.tensor_tensor(
                out=prod[:], in0=x_t[:], in1=wr_bc[:], op=mybir.AluOpType.mult
            )
            logit = vec_pool.tile([128, 1], FP32, tag="logit")
            nc.vector.reduce_sum(out=logit[:], in_=prod[:], axis=mybir.AxisListType.X)
            nc.vector.tensor_tensor(
                out=mask_nm[:, n:n + 1], in0=logit[:], in1=nbr_bc[:],
                op=mybir.AluOpType.is_gt,
            )
            nc.vector.tensor_copy(mask_u[:, n:n + 1], mask_nm[:, n:n + 1])

        # --- frac count: ones^T @ mask -> [1, NTT] -> reduce ---
        cnt_ps = pc.tile([1, NTT], FP32, tag="cnt_ps")
        nc.tensor.matmul(cnt_ps[:], lhsT=ones_col[:], rhs=mask_nm[:], start=True, stop=True)
        cnt_sb = const.tile([1, 1], FP32, tag="cnt_sb")
        nc.vector.reduce_sum(out=cnt_sb[:], in_=cnt_ps[:], axis=mybir.AxisListType.X)
        nc.sync.dma_start(out=cnt_ext[:], in_=cnt_sb[:])

        # --- MLP in chunks of NCH tokens ---
        for ch in range(NCHUNK):
            c0 = ch * NCH
            # moving operand for mm1: x^T fp16 slice [D, NCH]
            if ch == 0:
                xt_sb = xt_ch0
            else:
                xt_sb = []
                for k in range(KD):
                    t_ = xt_pool.tile([128, NCH], FP16, tag=f"xt_{k}")
                    nc.sync.dma_start(
                        out=t_[:], in_=xt16_in[k * 128:(k + 1) * 128, c0:c0 + NCH]
                    )
                    xt_sb.append(t_)
            # mm1: h^T[m-tile] [128, NCH] = relu(W1^T x^T + b1)
            h_sb = []
            for m in range(MH):
                ps = p1.tile([128, NCH], FP32, tag="ps1")
                for k in range(KD):
                    nc.tensor.matmul(
                        ps[:],
                        lhsT=w1_sb[k][:, m * 128:(m + 1) * 128],
                        rhs=xt_sb[k][:],
                        start=(k == 0),
                        stop=(k == KD - 1),
                    )
                h_m = h_pool.tile([128, NCH], FP16, tag=f"h_{m}")
                nc.scalar.activation(
                    h_m[:], ps[:], mybir.ActivationFunctionType.Relu,
                    bias=b1_sb[:, m:m + 1],
                )
                h_sb.append(h_m)
            # mm2: proc[token-tile] [128, D] = h W2 + b2, then select vs x
            for mt in range(NCHT):
                nt = ch * NCHT + mt
                out_t = out_pool.tile([128, D], FP32, tag="out_t")
                x_t2 = xrt_pool.tile([128, D], FP32, tag="x_sel")
                nc.sync.dma_start(out=x_t2[:], in_=x_in[nt * 128:(nt + 1) * 128, :])
                nc.vector.tensor_copy(out_t[:], x_t2[:])
                for ncol in range(D // 512):
                    ps2 = p2.tile([128, 512], FP32, tag="ps2")
                    for k in range(KH):
                        nc.tensor.matmul(
                            ps2[:],
                            lhsT=h_sb[k][:, mt * 128:(mt + 1) * 128],
                            rhs=w2_sb[k][:, ncol * 512:(ncol + 1) * 512],
                            start=(k == 0),
                            stop=(k == KH - 1),
                        )
                    # proc = psum + b2, in place in PSUM
                    nc.vector.tensor_tensor(
                        out=ps2[:],
                        in0=ps2[:],
                        in1=b2_bc[:, ncol * 512:(ncol + 1) * 512],
                        op=mybir.AluOpType.add,
                    )
                    nc.vector.copy_predicated(
                        out_t[:, ncol * 512:(ncol + 1) * 512],
                        mask_u[:, nt:nt + 1].to_broadcast([128, 512]),
                        ps2[:],
                    )
                nc.sync.dma_start(
                    out=out_ext[nt * 128:(nt + 1) * 128, :], in_=out_t[:]
                )

    nc.compile()
    return nc


def _host_prep(x, W_r, b_r, W1, b1, W2, b2):
    """Shard + precompute per-core input maps (host side, numpy only)."""
    xf = np.ascontiguousarray(np.asarray(x, dtype=np.float32).reshape(-1, D))
    w1_16 = np.ascontiguousarray(np.asarray(W1, dtype=np.float16))
    w2_16 = np.ascontiguousarray(np.asarray(W2, dtype=np.float16))
    b1f = np.ascontiguousarray(np.asarray(b1, dtype=np.float32).reshape(H))
    b2f = np.ascontiguousarray(np.asarray(b2, dtype=np.float32).reshape(D))
    wrf = np.ascontiguousarray(np.asarray(W_r, dtype=np.float32).reshape(D))
    brf = np.ascontiguousarray(np.asarray(b_r, dtype=np.float32).reshape(1))
    in_maps = []
    for c in range(N_CORES):
        xs = np.ascontiguousarray(xf[c * T:(c + 1) * T])
        xt16 = np.ascontiguousarray(xs.T.astype(np.float16))
        in_maps.append({
            "x": xs, "xt16": xt16, "w1": w1_16, "w2": w2_16,
            "b1": b1f, "b2": b2f, "wr": wrf, "br": brf,
        })
    return in_maps


_CACHED = {}


def _get_program():
    if "nc" not in _CACHED:
        _CACHED["nc"] = build_dense()
    return _CACHED["nc"]


def _get_runner():
    """Build the jitted 8-core executable once; reuse across kernel() calls."""
    if "runner" in _CACHED:
        return _CACHED["runner"]
    import jax
    import jax.numpy as jnp  # noqa: F401
    from jax.sharding import Mesh, PartitionSpec
    from jax.experimental.shard_map import shard_map
    from concourse import bass2jax, mybir as mb

    nc = _get_program()
    bass2jax.install_neuronx_cc_hook()

    partition_name = nc.partition_id_tensor.name if nc.partition_id_tensor else None
    in_names, out_names, out_avals, zero_shapes = [], [], [], []
    for alloc in nc.m.functions[0].allocations:
        if not isinstance(alloc, mb.MemoryLocationSet):
            continue
        name = alloc.memorylocations[0].name
        if alloc.kind == "ExternalInput":
            if name != partition_name:
                in_names.append(name)
        elif alloc.kind == "ExternalOutput":
            out_names.append(name)
            shape = tuple(alloc.tensor_shape)
            dtype = mb.dt.np(alloc.dtype)
            out_avals.append(jax.core.ShapedArray(shape, dtype))
            zero_shapes.append((shape, dtype))
    n_params = len(in_names)
    n_outs = len(out_names)
    all_in_names = list(in_names) + list(out_names)
    if partition_name is not None:
        all_in_names = all_in_names + [partition_name]

    def _body(*args):
        operands = list(args)
        if partition_name is not None:
            operands.append(bass2jax.partition_id_tensor())
        outs = bass2jax._bass_exec_p.bind(
            *operands,
            out_avals=tuple(out_avals),
            in_names=tuple(all_in_names),
            out_names=tuple(out_names),
            lowering_input_output_aliases=(),
            sim_require_finite=True,
            sim_require_nnan=True,
            nc=nc,
        )
        return tuple(outs)

    devices = jax.devices()[:N_CORES]
    mesh = Mesh(np.asarray(devices), ("core",))
    donate = tuple(range(n_params, n_params + n_outs))
    sharded = jax.jit(
        shard_map(
            _body, mesh=mesh,
            in_specs=(PartitionSpec("core"),) * (n_params + n_outs),
            out_specs=(PartitionSpec("core"),) * n_outs,
            check_rep=False,
        ),
        donate_argnums=donate,
        keep_unused=True,
    )
    _CACHED["runner"] = (sharded, in_names, out_names, zero_shapes)
    return _CACHED["runner"]


def _run(in_maps):
    sharded, in_names, out_names, zero_shapes = _get_runner()
    concat_in = [
        np.concatenate([np.asarray(in_maps[c][nm]) for c in range(N_CORES)], axis=0)
        for nm in in_names
    ]
    concat_zeros = [
        np.zeros((N_CORES * s[0], *s[1:]), dt) for (s, dt) in zero_shapes
    ]
    out_arrs = sharded(*concat_in, *concat_zeros)
    res = []
    for c in range(N_CORES):
        d = {}
        for i, nm in enumerate(out_names):
            a = np.asarray(out_arrs[i])
            per = a.shape[0] // N_CORES
            d[nm] = a[c * per:(c + 1) * per]
        res.append(d)
    return res


def kernel(x, W_r, b_r, W1, b1, W2, b2):
    in_maps = _host_prep(x, W_r, b_r, W1, b1, W2, b2)
    res = _run(in_maps)
    out = np.concatenate([res[c]["out"] for c in range(N_CORES)], axis=0)
    out = out.reshape(4, 4096, D)
    cnt = sum(float(res[c]["cnt"][0, 0]) for c in range(N_CORES))
    frac = np.float32(cnt / (N_CORES * T))
    return out, frac


# revision 14
# speedup vs baseline: 1.0052x; 1.0052x over previous
"""MoD (mixture-of-depths) routing layer on 8 Trainium2 NeuronCores.

Reference computation (per token t of x[B=4, S=4096, D=1024]):
    logit[t] = x[t] @ W_r + b_r
    mask[t]  = sigmoid(logit[t]) > 0.5      (== logit[t] > 0)
    h[t]     = relu(x[t] @ W1 + b1)
    proc[t]  = h[t] @ W2 + b2
    out[t]   = mask[t] ? proc[t] : x[t]
    frac     = mean(mask)

Sharding: tokens (B*S = 16384) are split contiguously across the 8 cores
(2048 tokens each); router + MLP weights are replicated.  No cross-core
communication is needed; frac is assembled on host from per-core counts.

Device kernel (per core, T = 2048 tokens):
  * router on DVE in fp32 (exact products; min |logit| over the dataset is
    ~1.6e-4 so fp16 routing would flip tokens, fp32 will not)
  * MLP matmuls on the PE array in fp16 (fp32 PSUM accumulation)
  * mm1 computes h^T tiles (stationary W1), mm2 consumes h^T as the
    stationary side so the output lands token-major; relu+b1 fused into the
    PSUM eviction, b2 added via a broadcast tile
  * final select is a copy of x overwritten by proc where mask!=0
"""

import sys
import numpy as np

sys.path.insert(0, "/opt/trn_rl_repo")

from contextlib import ExitStack  # noqa: E402

import concourse.bass as bass  # noqa: E402
import concourse.bacc as bacc  # noqa: E402
import concourse.mybir as mybir  # noqa: E402
import concourse.tile as tile  # noqa: E402

FP32 = mybir.dt.float32
FP16 = mybir.dt.float16

N_CORES = 8
T = 2048          # tokens per core
D = 1024          # model dim
H = 4096          # hidden dim
NTT = T // 128    # token tiles per core (16)
NCH = 256         # tokens per mm chunk
NCHT = NCH // 128  # token tiles per chunk (2)
NCHUNK = T // NCH  # chunks per core (8)
KD = D // 128     # k-tiles over model dim (8)
KH = H // 128     # k-tiles over hidden dim (32)
MH = H // 128     # m-tiles over hidden dim (32)


def build_dense():
    nc = bacc.Bacc()

    x_in = nc.declare_dram_parameter("x", [T, D], FP32, isOutput=False)
    xt16_in = nc.declare_dram_parameter("xt16", [D, T], FP16, isOutput=False)
    w1_in = nc.declare_dram_parameter("w1", [D, H], FP16, isOutput=False)
    w2_in = nc.declare_dram_parameter("w2", [H, D], FP16, isOutput=False)
    b1_in = nc.declare_dram_parameter("b1", [H], FP32, isOutput=False)
    b2_in = nc.declare_dram_parameter("b2", [D], FP32, isOutput=False)
    wr_in = nc.declare_dram_parameter("wr", [D], FP32, isOutput=False)
    br_in = nc.declare_dram_parameter("br", [1], FP32, isOutput=False)
    out_ext = nc.declare_dram_parameter("out", [T, D], FP32, isOutput=True)
    cnt_ext = nc.declare_dram_parameter("cnt", [1, 1], FP32, isOutput=True)

    with tile.TileContext(nc) as tc, ExitStack() as ctx:
        const = ctx.enter_context(tc.tile_pool(name="const", bufs=1))
        wts = ctx.enter_context(tc.tile_pool(name="wts", bufs=1))
        xt_pool = ctx.enter_context(tc.tile_pool(name="xt", bufs=2))
        h_pool = ctx.enter_context(tc.tile_pool(name="h", bufs=1))
        xrt_pool = ctx.enter_context(tc.tile_pool(name="xrt", bufs=2))
        vec_pool = ctx.enter_context(tc.tile_pool(name="vec", bufs=2))
        out_pool = ctx.enter_context(tc.tile_pool(name="outp", bufs=2))
        p1 = ctx.enter_context(tc.tile_pool(name="p1", bufs=3, space="PSUM"))
        p2 = ctx.enter_context(tc.tile_pool(name="p2", bufs=4, space="PSUM"))
        pc = ctx.enter_context(tc.tile_pool(name="pc", bufs=1, space="PSUM"))

        # --- constants / broadcasts ---
        wr_bc = const.tile([128, D], FP32, tag="wr_bc")
        nc.sync.dma_start(out=wr_bc[:], in_=wr_in[None, :].partition_broadcast(128))
        b2_bc = const.tile([128, D], FP32, tag="b2_bc")
        nc.sync.dma_start(out=b2_bc[:], in_=b2_in[None, :].partition_broadcast(128))
        nbr_bc = const.tile([128, 1], FP32, tag="nbr_bc")
        nc.sync.dma_start(out=nbr_bc[:], in_=br_in[None, :].partition_broadcast(128))
        nc.vector.tensor_scalar_mul(nbr_bc[:], nbr_bc[:], -1.0)
        b1_sb = const.tile([128, MH], FP32, tag="b1_sb")
        nc.sync.dma_start(out=b1_sb[:], in_=b1_in.rearrange("(m p) -> p m", p=128))
        ones_col = const.tile([128, 1], FP32, tag="ones")
        nc.any.memset(ones_col[:], 1.0)

        # --- first chunk's activations, then W1, then W2 (W2 on the ACT
        # HWDGE queue so it doesn't delay the PE's first matmuls) ---
        xt_ch0 = []
        for k in range(KD):
            t_ = xt_pool.tile([128, NCH], FP16, tag=f"xt_{k}")
            nc.sync.dma_start(out=t_[:], in_=xt16_in[k * 128:(k + 1) * 128, 0:NCH])
            xt_ch0.append(t_)
        w1_sb = []
        for k in range(KD):
            t_ = wts.tile([128, H], FP16, tag=f"w1_{k}")
            nc.sync.dma_start(out=t_[:], in_=w1_in[k * 128:(k + 1) * 128, :])
            w1_sb.append(t_)
        w2_sb = []
        for k in range(KH):
            t_ = wts.tile([128, D], FP16, tag=f"w2_{k}")
            nc.sync.dma_start(out=t_[:], in_=w2_in[k * 128:(k + 1) * 128, :])
            w2_sb.append(t_)

        # --- router: fp32 on DVE, token-major ---
        mask_nm = const.tile([128, NTT], FP32, tag="mask")
        mask_u = const.tile([128, NTT], mybir.dt.uint32, tag="mask_u")
        for n in range(NTT):
            x_t = xrt_pool.tile([128, D], FP32, tag="x_rt")
            nc.scalar.dma_start(out=x_t[:], in_=x_in[n * 128:(n + 1) * 128, :])
            prod = vec_pool.tile([128, D], FP32, tag="prod")
            nc.vector.tensor_tensor(
                out=prod[:], in0=x_t[:], in1=wr_bc[:], op=mybir.AluOpType.mult
            )
            logit = vec_pool.tile([128, 1], FP32, tag="logit")
            nc.vector.reduce_sum(out=logit[:], in_=prod[:], axis=mybir.AxisListType.X)
            nc.vector.tensor_tensor(
                out=mask_nm[:, n:n + 1], in0=logit[:], in1=nbr_bc[:],
                op=mybir.AluOpType.is_gt,
            )
            nc.vector.tensor_copy(mask_u[:, n:n + 1], mask_nm[:, n:n + 1])

        # --- frac count: ones^T @ mask -> [1, NTT] -> reduce ---
        cnt_ps = pc.tile([1, NTT], FP32, tag="cnt_ps")
        nc.tensor.matmul(cnt_ps[:], lhsT=ones_col[:], rhs=mask_nm[:], start=True, stop=True)
        cnt_sb = const.tile([1, 1], FP32, tag="cnt_sb")
        nc.vector.reduce_sum(out=cnt_sb[:], in_=cnt_ps[:], axis=mybir.AxisListType.X)
        nc.sync.dma_start(out=cnt_ext[:], in_=cnt_sb[:])

        # --- MLP in chunks of NCH tokens ---
        for ch in range(NCHUNK):
            c0 = ch * NCH
            # moving operand for mm1: x^T fp16 slice [D, NCH]
            if ch == 0:
                xt_sb = xt_ch0
            else:
                xt_sb = []
                for k in range(KD):
                    t_ = xt_pool.tile([128, NCH], FP16, tag=f"xt_{k}")
                    nc.sync.dma_start(
                        out=t_[:], in_=xt16_in[k * 128:(k + 1) * 128, c0:c0 + NCH]
                    )
                    xt_sb.append(t_)
            # mm1: h^T[m-tile] [128, NCH] = relu(W1^T x^T + b1)
            h_sb = []
            for m in range(MH):
                ps = p1.tile([128, NCH], FP32, tag="ps1")
                for k in range(KD):
                    nc.tensor.matmul(
                        ps[:],
                        lhsT=w1_sb[k][:, m * 128:(m + 1) * 128],
                        rhs=xt_sb[k][:],
                        start=(k == 0),
                        stop=(k == KD - 1),
                    )
                h_m = h_pool.tile([128, NCH], FP16, tag=f"h_{m}")
                nc.scalar.activation(
                    h_m[:], ps[:], mybir.ActivationFunctionType.Relu,
                    bias=b1_sb[:, m:m + 1],
                )
                h_sb.append(h_m)
            # mm2: proc[token-tile] [128, D] = h W2 + b2, then select vs x
            for mt in range(NCHT):
                nt = ch * NCHT + mt
                out_t = out_pool.tile([128, D], FP32, tag="out_t")
                x_t2 = xrt_pool.tile([128, D], FP32, tag="x_sel")
                nc.sync.dma_start(out=x_t2[:], in_=x_in[nt * 128:(nt + 1) * 128, :])
                nc.vector.tensor_copy(out_t[:], x_t2[:])
                for ncol in range(D // 512):
                    ps2 = p2.tile([128, 512], FP32, tag="ps2")
                    for k in range(KH):
                        nc.tensor.matmul(
                            ps2[:],
                            lhsT=h_sb[k][:, mt * 128:(mt + 1) * 128],
                            rhs=w2_sb[k][:, ncol * 512:(ncol + 1) * 512],
                            start=(k == 0),
                            stop=(k == KH - 1),
                        )
                    # proc = psum + b2, in place in PSUM
                    nc.vector.tensor_tensor(
                        out=ps2[:],
                        in0=ps2[:],
                        in1=b2_bc[:, ncol * 512:(ncol + 1) * 512],
                        op=mybir.AluOpType.add,
                    )
                    nc.vector.copy_predicated(
                        out_t[:, ncol * 512:(ncol + 1) * 512],
                        mask_u[:, nt:nt + 1].to_broadcast([128, 512]),
                        ps2[:],
                    )
                nc.sync.dma_start(
                    out=out_ext[nt * 128:(nt + 1) * 128, :], in_=out_t[:]
                )

    nc.compile()
    return nc


def _host_prep(x, W_r, b_r, W1, b1, W2, b2):
    """Shard + precompute per-core input maps (host side, numpy only)."""
    xf = np.ascontiguousarray(np.asarray(x, dtype=np.float32).reshape(-1, D))
    w1_16 = np.ascontiguousarray(np.asarray(W1, dtype=np.float16))
    w2_16 = np.ascontiguousarray(np.asarray(W2, dtype=np.float16))
    b1f = np.ascontiguousarray(np.asarray(b1, dtype=np.float32).reshape(H))
    b2f = np.ascontiguousarray(np.asarray(b2, dtype=np.float32).reshape(D))
    wrf = np.ascontiguousarray(np.asarray(W_r, dtype=np.float32).reshape(D))
    brf = np.ascontiguousarray(np.asarray(b_r, dtype=np.float32).reshape(1))
    in_maps = []
    for c in range(N_CORES):
        xs = np.ascontiguousarray(xf[c * T:(c + 1) * T])
        xt16 = np.ascontiguousarray(xs.T.astype(np.float16))
        in_maps.append({
            "x": xs, "xt16": xt16, "w1": w1_16, "w2": w2_16,
            "b1": b1f, "b2": b2f, "wr": wrf, "br": brf,
        })
    return in_maps


_CACHED = {}


def _get_program():
    if "nc" not in _CACHED:
        _CACHED["nc"] = build_dense()
    return _CACHED["nc"]


def _get_runner():
    """Build the jitted 8-core executable once; reuse across kernel() calls."""
    if "runner" in _CACHED:
        return _CACHED["runner"]
    import jax
    import jax.numpy as jnp  # noqa: F401
    from jax.sharding import Mesh, PartitionSpec
    from jax.experimental.shard_map import shard_map
    from concourse import bass2jax, mybir as mb

    nc = _get_program()
    bass2jax.install_neuronx_cc_hook()

    partition_name = nc.partition_id_tensor.name if nc.partition_id_tensor else None
    in_names, out_names, out_avals, zero_shapes = [], [], [], []
    for alloc in nc.m.functions[0].allocations:
        if not isinstance(alloc, mb.MemoryLocationSet):
            continue
        name = alloc.memorylocations[0].name
        if alloc.kind == "ExternalInput":
            if name != partition_name:
                in_names.append(name)
        elif alloc.kind == "ExternalOutput":
            out_names.append(name)
            shape = tuple(alloc.tensor_shape)
            dtype = mb.dt.np(alloc.dtype)
            out_avals.append(jax.core.ShapedArray(shape, dtype))
            zero_shapes.append((shape, dtype))
    n_params = len(in_names)
    n_outs = len(out_names)
    all_in_names = list(in_names) + list(out_names)
    if partition_name is not None:
        all_in_names = all_in_names + [partition_name]

    def _body(*args):
        operands = list(args)
        if partition_name is not None:
            operands.append(bass2jax.partition_id_tensor())
        outs = bass2jax._bass_exec_p.bind(
            *operands,
            out_avals=tuple(out_avals),
            in_names=tuple(all_in_names),
            out_names=tuple(out_names),
            lowering_input_output_aliases=(),
            sim_require_finite=True,
            sim_require_nnan=True,
            nc=nc,
        )
        return tuple(outs)

    devices = jax.devices()[:N_CORES]
    mesh = Mesh(np.asarray(devices), ("core",))
    donate = tuple(range(n_params, n_params + n_outs))
    sharded = jax.jit(
        shard_map(
            _body, mesh=mesh,
            in_specs=(PartitionSpec("core"),) * (n_params + n_outs),
            out_specs=(PartitionSpec("core"),) * n_outs,
            check_rep=False,
        ),
        donate_argnums=donate,
        keep_unused=True,
    )
    _CACHED["runner"] = (sharded, in_names, out_names, zero_shapes)
    return _CACHED["runner"]


def _run(in_maps):
    sharded, in_names, out_names, zero_shapes = _get_runner()
    concat_in = [
        np.concatenate([np.asarray(in_maps[c][nm]) for c in range(N_CORES)], axis=0)
        for nm in in_names
    ]
    concat_zeros = [
        np.zeros((N_CORES * s[0], *s[1:]), dt) for (s, dt) in zero_shapes
    ]
    out_arrs = sharded(*concat_in, *concat_zeros)
    res = []
    for c in range(N_CORES):
        d = {}
        for i, nm in enumerate(out_names):
            a = np.asarray(out_arrs[i])
            per = a.shape[0] // N_CORES
            d[nm] = a[c * per:(c + 1) * per]
        res.append(d)
    return res


def kernel(x, W_r, b_r, W1, b1, W2, b2):
    in_maps = _host_prep(x, W_r, b_r, W1, b1, W2, b2)
    res = _run(in_maps)
    out = np.concatenate([res[c]["out"] for c in range(N_CORES)], axis=0)
    out = out.reshape(4, 4096, D)
    cnt = sum(float(res[c]["cnt"][0, 0]) for c in range(N_CORES))
    frac = np.float32(cnt / (N_CORES * T))
    return out, frac


# revision 17
# speedup vs baseline: 1.1400x; 1.1341x over previous
"""MoD (mixture-of-depths) routing layer on 8 Trainium2 NeuronCores.

Reference computation (per token t of x[B=4, S=4096, D=1024]):
    logit[t] = x[t] @ W_r + b_r
    mask[t]  = sigmoid(logit[t]) > 0.5      (== logit[t] > 0)
    h[t]     = relu(x[t] @ W1 + b1)
    proc[t]  = h[t] @ W2 + b2
    out[t]   = mask[t] ? proc[t] : x[t]
    frac     = mean(mask)

Sharding: tokens (B*S = 16384) are split contiguously across the 8 cores
(2048 tokens each); router + MLP weights are replicated.  No cross-core
communication is needed; frac is assembled on host from per-core counts.

Device kernel (per core, T = 2048 tokens):
  * router on DVE in fp32 (exact products; min |logit| over the dataset is
    ~1.6e-4 so fp16 routing would flip tokens, fp32 will not)
  * MLP matmuls on the PE array in fp16 (fp32 PSUM accumulation)
  * mm1 computes h^T tiles (stationary W1), mm2 consumes h^T as the
    stationary side so the output lands token-major; relu+b1 fused into the
    PSUM eviction, b2 added via a broadcast tile
  * final select is a copy of x overwritten by proc where mask!=0
"""

import sys
import numpy as np

sys.path.insert(0, "/opt/trn_rl_repo")

from contextlib import ExitStack  # noqa: E402

import concourse.bass as bass  # noqa: E402
import concourse.bacc as bacc  # noqa: E402
import concourse.mybir as mybir  # noqa: E402
import concourse.tile as tile  # noqa: E402

FP32 = mybir.dt.float32
FP16 = mybir.dt.float16

N_CORES = 8
T = 2048          # tokens per core
D = 1024          # model dim
H = 4096          # hidden dim
NTT = T // 128    # token tiles per core (16)
NCH = 256         # tokens per mm chunk
NCHT = NCH // 128  # token tiles per chunk (2)
NCHUNK = T // NCH  # chunks per core (8)
KD = D // 128     # k-tiles over model dim (8)
KH = H // 128     # k-tiles over hidden dim (32)
MH = H // 128     # m-tiles over hidden dim (32)


def build_dense():
    nc = bacc.Bacc()

    x_in = nc.declare_dram_parameter("x", [T, D], FP32, isOutput=False)
    xt16_in = nc.declare_dram_parameter("xt16", [D, T], FP16, isOutput=False)
    w1_in = nc.declare_dram_parameter("w1", [D, H], FP16, isOutput=False)
    w2_in = nc.declare_dram_parameter("w2", [H, D], FP16, isOutput=False)
    b1_in = nc.declare_dram_parameter("b1", [H], FP32, isOutput=False)
    b2_in = nc.declare_dram_parameter("b2", [D], FP32, isOutput=False)
    wr_in = nc.declare_dram_parameter("wr", [D], FP32, isOutput=False)
    br_in = nc.declare_dram_parameter("br", [1], FP32, isOutput=False)
    out_ext = nc.declare_dram_parameter("out", [T, D], FP32, isOutput=True)
    cnt_ext = nc.declare_dram_parameter("cnt", [1, 1], FP32, isOutput=True)

    with tile.TileContext(nc) as tc, ExitStack() as ctx:
        const = ctx.enter_context(tc.tile_pool(name="const", bufs=1))
        wts = ctx.enter_context(tc.tile_pool(name="wts", bufs=1))
        xt_pool = ctx.enter_context(tc.tile_pool(name="xt", bufs=2))
        h_pool = ctx.enter_context(tc.tile_pool(name="h", bufs=1))
        xrt_pool = ctx.enter_context(tc.tile_pool(name="xrt", bufs=2))
        vec_pool = ctx.enter_context(tc.tile_pool(name="vec", bufs=2))
        out_pool = ctx.enter_context(tc.tile_pool(name="outp", bufs=2))
        p1 = ctx.enter_context(tc.tile_pool(name="p1", bufs=3, space="PSUM"))
        p2 = ctx.enter_context(tc.tile_pool(name="p2", bufs=4, space="PSUM"))
        pc = ctx.enter_context(tc.tile_pool(name="pc", bufs=1, space="PSUM"))

        # --- constants / broadcasts ---
        wr_bc = const.tile([128, D], FP32, tag="wr_bc")
        nc.sync.dma_start(out=wr_bc[:], in_=wr_in[None, :].partition_broadcast(128))
        b2_bc = const.tile([128, D], FP32, tag="b2_bc")
        nc.sync.dma_start(out=b2_bc[:], in_=b2_in[None, :].partition_broadcast(128))
        nbr_bc = const.tile([128, 1], FP32, tag="nbr_bc")
        nc.sync.dma_start(out=nbr_bc[:], in_=br_in[None, :].partition_broadcast(128))
        nc.vector.tensor_scalar_mul(nbr_bc[:], nbr_bc[:], -1.0)
        b1_sb = const.tile([128, MH], FP32, tag="b1_sb")
        nc.sync.dma_start(out=b1_sb[:], in_=b1_in.rearrange("(m p) -> p m", p=128))
        ones_col = const.tile([128, 1], FP32, tag="ones")
        nc.any.memset(ones_col[:], 1.0)

        # --- first chunk's activations, then W1, then W2 (W2 on the ACT
        # HWDGE queue so it doesn't delay the PE's first matmuls) ---
        xt_ch0 = []
        for k in range(KD):
            t_ = xt_pool.tile([128, NCH], FP16, tag=f"xt_{k}")
            nc.sync.dma_start(out=t_[:], in_=xt16_in[k * 128:(k + 1) * 128, 0:NCH])
            xt_ch0.append(t_)
        w1_sb = []
        for k in range(KD):
            t_ = wts.tile([128, H], FP16, tag=f"w1_{k}")
            nc.sync.dma_start(out=t_[:], in_=w1_in[k * 128:(k + 1) * 128, :])
            w1_sb.append(t_)
        w2_sb = []
        for k in range(KH):
            t_ = wts.tile([128, D], FP16, tag=f"w2_{k}")
            nc.sync.dma_start(out=t_[:], in_=w2_in[k * 128:(k + 1) * 128, :])
            w2_sb.append(t_)

        # --- router: fp32 on DVE, token-major ---
        mask_nm = const.tile([128, NTT], FP32, tag="mask")
        mask_u = const.tile([128, NTT], mybir.dt.uint32, tag="mask_u")
        for n in range(NTT):
            x_t = xrt_pool.tile([128, D], FP32, tag="x_rt")
            nc.scalar.dma_start(out=x_t[:], in_=x_in[n * 128:(n + 1) * 128, :])
            prod = vec_pool.tile([128, D], FP32, tag="prod")
            nc.vector.tensor_tensor(
                out=prod[:], in0=x_t[:], in1=wr_bc[:], op=mybir.AluOpType.mult
            )
            logit = vec_pool.tile([128, 1], FP32, tag="logit")
            nc.vector.reduce_sum(out=logit[:], in_=prod[:], axis=mybir.AxisListType.X)
            nc.vector.tensor_tensor(
                out=mask_nm[:, n:n + 1], in0=logit[:], in1=nbr_bc[:],
                op=mybir.AluOpType.is_gt,
            )
            nc.vector.tensor_copy(mask_u[:, n:n + 1], mask_nm[:, n:n + 1])

        # --- frac count: ones^T @ mask -> [1, NTT] -> reduce ---
        cnt_ps = pc.tile([1, NTT], FP32, tag="cnt_ps")
        nc.tensor.matmul(cnt_ps[:], lhsT=ones_col[:], rhs=mask_nm[:], start=True, stop=True)
        cnt_sb = const.tile([1, 1], FP32, tag="cnt_sb")
        nc.vector.reduce_sum(out=cnt_sb[:], in_=cnt_ps[:], axis=mybir.AxisListType.X)
        nc.gpsimd.dma_start(out=cnt_ext[:], in_=cnt_sb[:])

        # --- MLP in chunks of NCH tokens ---
        for ch in range(NCHUNK):
            c0 = ch * NCH
            # moving operand for mm1: x^T fp16 slice [D, NCH]
            if ch == 0:
                xt_sb = xt_ch0
            else:
                xt_sb = []
                for k in range(KD):
                    t_ = xt_pool.tile([128, NCH], FP16, tag=f"xt_{k}")
                    nc.sync.dma_start(
                        out=t_[:], in_=xt16_in[k * 128:(k + 1) * 128, c0:c0 + NCH]
                    )
                    xt_sb.append(t_)
            # mm1: h^T[m-tile] [128, NCH] = relu(W1^T x^T + b1)
            h_sb = []
            for m in range(MH):
                ps = p1.tile([128, NCH], FP32, tag="ps1")
                for k in range(KD):
                    nc.tensor.matmul(
                        ps[:],
                        lhsT=w1_sb[k][:, m * 128:(m + 1) * 128],
                        rhs=xt_sb[k][:],
                        start=(k == 0),
                        stop=(k == KD - 1),
                    )
                h_m = h_pool.tile([128, NCH], FP16, tag=f"h_{m}")
                nc.scalar.activation(
                    h_m[:], ps[:], mybir.ActivationFunctionType.Relu,
                    bias=b1_sb[:, m:m + 1],
                )
                h_sb.append(h_m)
            # mm2: proc[token-tile] [128, D] = h W2 + b2, then select vs x
            for mt in range(NCHT):
                nt = ch * NCHT + mt
                out_t = out_pool.tile([128, D], FP32, tag="out_t")
                x_t2 = xrt_pool.tile([128, D], FP32, tag="x_sel")
                nc.scalar.dma_start(out=x_t2[:], in_=x_in[nt * 128:(nt + 1) * 128, :])
                nc.vector.tensor_copy(out_t[:], x_t2[:])
                for ncol in range(D // 512):
                    ps2 = p2.tile([128, 512], FP32, tag="ps2")
                    for k in range(KH):
                        nc.tensor.matmul(
                            ps2[:],
                            lhsT=h_sb[k][:, mt * 128:(mt + 1) * 128],
                            rhs=w2_sb[k][:, ncol * 512:(ncol + 1) * 512],
                            start=(k == 0),
                            stop=(k == KH - 1),
                        )
                    # proc = psum + b2, in place in PSUM
                    nc.vector.tensor_tensor(
                        out=ps2[:],
                        in0=ps2[:],
                        in1=b2_bc[:, ncol * 512:(ncol + 1) * 512],
                        op=mybir.AluOpType.add,
                    )
                    nc.vector.copy_predicated(
                        out_t[:, ncol * 512:(ncol + 1) * 512],
                        mask_u[:, nt:nt + 1].to_broadcast([128, 512]),
                        ps2[:],
                    )
                nc.gpsimd.dma_start(
                    out=out_ext[nt * 128:(nt + 1) * 128, :], in_=out_t[:]
                )

    nc.compile()
    return nc


def _host_prep(x, W_r, b_r, W1, b1, W2, b2):
    """Shard + precompute per-core input maps (host side, numpy only)."""
    xf = np.ascontiguousarray(np.asarray(x, dtype=np.float32).reshape(-1, D))
    w1_16 = np.ascontiguousarray(np.asarray(W1, dtype=np.float16))
    w2_16 = np.ascontiguousarray(np.asarray(W2, dtype=np.float16))
    b1f = np.ascontiguousarray(np.asarray(b1, dtype=np.float32).reshape(H))
    b2f = np.ascontiguousarray(np.asarray(b2, dtype=np.float32).reshape(D))
    wrf = np.ascontiguousarray(np.asarray(W_r, dtype=np.float32).reshape(D))
    brf = np.ascontiguousarray(np.asarray(b_r, dtype=np.float32).reshape(1))
    in_maps = []
    for c in range(N_CORES):
        xs = np.ascontiguousarray(xf[c * T:(c + 1) * T])
        xt16 = np.ascontiguousarray(xs.T.astype(np.float16))
        in_maps.append({
            "x": xs, "xt16": xt16, "w1": w1_16, "w2": w2_16,
            "b1": b1f, "b2": b2f, "wr": wrf, "br": brf,
        })
    return in_maps


_CACHED = {}


def _get_program():
    if "nc" not in _CACHED:
        _CACHED["nc"] = build_dense()
    return _CACHED["nc"]


def _get_runner():
    """Build the jitted 8-core executable once; reuse across kernel() calls."""
    if "runner" in _CACHED:
        return _CACHED["runner"]
    import jax
    import jax.numpy as jnp  # noqa: F401
    from jax.sharding import Mesh, PartitionSpec
    from jax.experimental.shard_map import shard_map
    from concourse import bass2jax, mybir as mb

    nc = _get_program()
    bass2jax.install_neuronx_cc_hook()

    partition_name = nc.partition_id_tensor.name if nc.partition_id_tensor else None
    in_names, out_names, out_avals, zero_shapes = [], [], [], []
    for alloc in nc.m.functions[0].allocations:
        if not isinstance(alloc, mb.MemoryLocationSet):
            continue
        name = alloc.memorylocations[0].name
        if alloc.kind == "ExternalInput":
            if name != partition_name:
                in_names.append(name)
        elif alloc.kind == "ExternalOutput":
            out_names.append(name)
            shape = tuple(alloc.tensor_shape)
            dtype = mb.dt.np(alloc.dtype)
            out_avals.append(jax.core.ShapedArray(shape, dtype))
            zero_shapes.append((shape, dtype))
    n_params = len(in_names)
    n_outs = len(out_names)
    all_in_names = list(in_names) + list(out_names)
    if partition_name is not None:
        all_in_names = all_in_names + [partition_name]

    def _body(*args):
        operands = list(args)
        if partition_name is not None:
            operands.append(bass2jax.partition_id_tensor())
        outs = bass2jax._bass_exec_p.bind(
            *operands,
            out_avals=tuple(out_avals),
            in_names=tuple(all_in_names),
            out_names=tuple(out_names),
            lowering_input_output_aliases=(),
            sim_require_finite=True,
            sim_require_nnan=True,
            nc=nc,
        )
        return tuple(outs)

    devices = jax.devices()[:N_CORES]
    mesh = Mesh(np.asarray(devices), ("core",))
    donate = tuple(range(n_params, n_params + n_outs))
    sharded = jax.jit(
        shard_map(
            _body, mesh=mesh,
            in_specs=(PartitionSpec("core"),) * (n_params + n_outs),
            out_specs=(PartitionSpec("core"),) * n_outs,
            check_rep=False,
        ),
        donate_argnums=donate,
        keep_unused=True,
    )
    _CACHED["runner"] = (sharded, in_names, out_names, zero_shapes)
    return _CACHED["runner"]


def _run(in_maps):
    sharded, in_names, out_names, zero_shapes = _get_runner()
    concat_in = [
        np.concatenate([np.asarray(in_maps[c][nm]) for c in range(N_CORES)], axis=0)
        for nm in in_names
    ]
    concat_zeros = [
        np.zeros((N_CORES * s[0], *s[1:]), dt) for (s, dt) in zero_shapes
    ]
    out_arrs = sharded(*concat_in, *concat_zeros)
    res = []
    for c in range(N_CORES):
        d = {}
        for i, nm in enumerate(out_names):
            a = np.asarray(out_arrs[i])
            per = a.shape[0] // N_CORES
            d[nm] = a[c * per:(c + 1) * per]
        res.append(d)
    return res


def kernel(x, W_r, b_r, W1, b1, W2, b2):
    in_maps = _host_prep(x, W_r, b_r, W1, b1, W2, b2)
    res = _run(in_maps)
    out = np.concatenate([res[c]["out"] for c in range(N_CORES)], axis=0)
    out = out.reshape(4, 4096, D)
    cnt = sum(float(res[c]["cnt"][0, 0]) for c in range(N_CORES))
    frac = np.float32(cnt / (N_CORES * T))
    return out, frac
